# revision 17
# baseline (speedup 1.0000x reference)
"""GatedGCN message-passing kernel for 8 Trainium2 NeuronCores (Bass/Tile).

Math (reference):
    newX = X @ Wn + bn
    agg  = segment_sum(a_vals[:,None] * newX[col], row, N)
    gate = sigmoid(X @ Wgi + bgi + agg @ Wgn + bgn)
    out  = agg * gate + X * (1 - gate)

Device strategy (per core, destination-sharded edges):
    Linearity lets the dense projection move past the aggregation:
        agg = (segsum(a * X_aug[col])) @ Wn_aug,    X_aug = [X | 1], Wn_aug = [Wn; bn]
    so the gather runs on raw bf16 X rows (SWDGE dma_gather, 256B rows) and the
    segment-sum is computed as one-hot matmuls accumulating in PSUM:
        per 128-edge chunk c of a 128-destination block b:
            S[e, d]    = (iota[d] == row_rel[e]) * a[e]          (DVE, 2 ops)
            aggX[:, b] += Xg_c.T @ S_c                            (PE, PSUM accum)
    giving aggX feature-major [97, dst]. All downstream dense math is then
        z    = Xo_aug @ Wgi_aug + aggX_aug @ (Wn_aug @ Wgn)      (biases folded)
        agg  = aggX_aug @ Wn_aug
        out  = agg * sigmoid(z) + Xo * sigmoid(-z)
    dma_gather uses int16 indices, so the node table is split in two 25000-row
    halves; per-half edge streams are block-major contiguous so every gather
    prep covers a contiguous index span.

Perf structure (v2):
      - gathers use prepare_only=True + trigger_dma: the Pool engine only
        writes descriptors (~0.34ns each) into a 2048-entry SWDGE ring
        (dynamic_dma_scratch_size=32768) over 4 queues; the 16 DMA engines
        drain asynchronously. This removes the Pool-engine hold that
        serialized the baseline.
      - the selection matrix S is built in [lane, dst, chunk] layout against a
        materialized iota3[P, DW, K] so every DVE operand has a packed
        (stride-1, 2-byte) last dim -> DVE 2x_1p mode (2 elem/cycle/lane).
      - resident loads ride the Sync engine, keeping Scalar for
        sigmoids + PSUM->SBUF copies.
"""

import os

# Reset NeuronCores on runtime open: residual device state from prior runs
# otherwise degrades kernel time by ~15%. Must be set before NRT init.
os.environ.setdefault("NEURON_RT_RESET_CORES", "1")

import numpy as np
import ml_dtypes

N = 50000
E = 800000
D = 96
DA = D + 1          # augmented features (ones column)
ROWE = 128          # padded row elements in the gather table (256B bf16)
NC_ = 8
NSH = N // NC_      # 6250 nodes per core
NHALF = N // 2      # table split for int16 gather indices
DW = 128            # destinations per block
NBLK = (NSH + DW - 1) // DW          # 49 blocks per core
NPAD = NBLK * DW                     # 6272 padded nodes per core
P = 128                              # edges per chunk
BPB = 7                              # blocks per batch
NBATCH = NBLK // BPB                 # 7 batches
MAXI = 1024                          # gather idxs per prep (SWDGE ring cap)

_bf16 = ml_dtypes.bfloat16

_prog_cache = {}


def _host_prep(X, a_vals, Wn, bn, Wgi, bgi, Wgn, bgn, row, col):
    X = np.asarray(X, np.float32)
    a_vals = np.asarray(a_vals, np.float32)
    row = np.asarray(row, np.int64)
    col = np.asarray(col, np.int64)

    Wn_aug = np.vstack([np.asarray(Wn, np.float32), np.asarray(bn, np.float32)[None, :]])
    Wgi_aug = np.vstack([np.asarray(Wgi, np.float32),
                         (np.asarray(bgi, np.float32) + np.asarray(bgn, np.float32))[None, :]])
    W2_aug = Wn_aug @ np.asarray(Wgn, np.float32)

    X_pad = np.zeros((N, ROWE), np.float32)
    X_pad[:, :D] = X
    X_pad[:, D] = 1.0
    X_pad_bf = X_pad.astype(_bf16)
    xaug0 = np.ascontiguousarray(X_pad_bf[:NHALF])
    xaug1 = np.ascontiguousarray(X_pad_bf[NHALF:])

    core = row // NSH
    local = row - core * NSH
    blk = local // DW
    rr = local - blk * DW
    half = (col >= NHALF).astype(np.int64)

    # group edges by (core, block, half); get within-group positions
    gkey = (core * NBLK + blk) * 2 + half
    order = np.argsort(gkey, kind="stable")
    gk_sorted = gkey[order]
    counts = np.bincount(gk_sorted, minlength=NC_ * NBLK * 2)
    starts = np.concatenate([[0], np.cumsum(counts)])
    pos = np.arange(E, dtype=np.int64) - starts[gk_sorted]

    cnt2 = counts.reshape(NC_ * NBLK, 2)
    C0 = int(np.ceil(cnt2[:, 0].max() / P))
    C1 = int(np.ceil(cnt2[:, 1].max() / P))

    # per-half slot: block-major chunks, so batches cover contiguous spans
    b_all = blk[order]
    h_all = half[order]
    c_all = pos // P
    lane_all = pos - c_all * P
    ch_all = np.where(h_all == 0, C0, C1)
    slot_all = (b_all * ch_all + c_all) * P + lane_all   # within-half slot

    col_l = (col - half * NHALF)[order].astype(np.int32)   # local table row
    rr_o = rr[order].astype(np.float32)
    av_o = a_vals[order].astype(np.float32)
    core_o = core[order]

    G0 = NBLK * C0
    G1 = NBLK * C1

    per_core = []
    for k in range(NC_):
        d = {}
        for h, G in ((0, G0), (1, G1)):
            m = (core_o == k) & (h_all == h)
            slot = slot_all[m]
            idx_arr = np.zeros(G * P, np.int32)
            rr_arr = np.zeros(G * P, np.float32)
            av_arr = np.zeros(G * P, np.float32)
            idx_arr[slot] = col_l[m]
            rr_arr[slot] = rr_o[m]
            av_arr[slot] = av_o[m]

            d[f"rr{h}"] = np.ascontiguousarray(rr_arr.reshape(G, P).T).astype(_bf16)
            d[f"av{h}"] = np.ascontiguousarray(av_arr.reshape(G, P).T).astype(_bf16)
            # gather index stream wrapped in 16 partitions, replicated x8
            ix = idx_arr.reshape(G, P).astype(np.int16)
            wr = ix.reshape(G * P // 16, 16).T                 # [16, G*8]
            d[f"ixw{h}"] = np.ascontiguousarray(np.tile(wr, (8, 1)))   # [128, G*8]

        xo = np.zeros((NPAD, DA), np.float32)
        xo[:NSH] = X_pad[k * NSH:(k + 1) * NSH, :DA]
        d["xofm"] = np.ascontiguousarray(xo.T).astype(_bf16)          # [97, 6272]
        d["xonm"] = np.ascontiguousarray(xo[:, :D])                   # [6272, 96] f32
        d["xaug0"] = xaug0
        d["xaug1"] = xaug1
        d["wgi"] = Wgi_aug.astype(_bf16)
        d["w2"] = W2_aug.astype(_bf16)
        d["wn"] = Wn_aug.astype(_bf16)
        per_core.append(d)
    return per_core, (C0, C1)


def _build_program(C0, C1):
    import concourse.bass as bass
    import concourse.tile as tile
    from concourse import bacc, mybir

    CPB = C0 + C1
    G0 = NBLK * C0
    G1 = NBLK * C1
    KMAX = max(C0, C1)

    # detect_race_conditions=False: the sem-reuse rule would force each prep
    # to wait for the previous same-queue DMA (serializing the gather); the
    # per-queue sems are monotonic counters and the fence targets are
    # cumulative, so reuse is sound. Ring backpressure paces the preps.
    nc = bacc.Bacc("TRN2", target_bir_lowering=False, debug=False, num_devices=NC_,
                   num_swdge_queues=4, detect_race_conditions=False)

    xaug0_d = nc.dram_tensor("xaug0", [NHALF, ROWE], mybir.dt.bfloat16, kind="ExternalInput")
    xaug1_d = nc.dram_tensor("xaug1", [NHALF, ROWE], mybir.dt.bfloat16, kind="ExternalInput")
    ixw0_d = nc.dram_tensor("ixw0", [P, G0 * 8], mybir.dt.int16, kind="ExternalInput")
    ixw1_d = nc.dram_tensor("ixw1", [P, G1 * 8], mybir.dt.int16, kind="ExternalInput")
    rr0_d = nc.dram_tensor("rr0", [P, G0], mybir.dt.bfloat16, kind="ExternalInput")
    rr1_d = nc.dram_tensor("rr1", [P, G1], mybir.dt.bfloat16, kind="ExternalInput")
    av0_d = nc.dram_tensor("av0", [P, G0], mybir.dt.bfloat16, kind="ExternalInput")
    av1_d = nc.dram_tensor("av1", [P, G1], mybir.dt.bfloat16, kind="ExternalInput")
    xofm_d = nc.dram_tensor("xofm", [DA, NPAD], mybir.dt.bfloat16, kind="ExternalInput")
    xonm_d = nc.dram_tensor("xonm", [NPAD, D], mybir.dt.float32, kind="ExternalInput")
    wgi_d = nc.dram_tensor("wgi", [DA, D], mybir.dt.bfloat16, kind="ExternalInput")
    w2_d = nc.dram_tensor("w2", [DA, D], mybir.dt.bfloat16, kind="ExternalInput")
    wn_d = nc.dram_tensor("wn", [DA, D], mybir.dt.bfloat16, kind="ExternalInput")
    y_d = nc.dram_tensor("y", [NPAD, D], mybir.dt.float32, kind="ExternalOutput")

    with tile.TileContext(nc) as tc:
        with (
            tc.tile_pool(name="const", bufs=1) as cpool,
            tc.tile_pool(name="xg0", bufs=2) as xg0p,
            tc.tile_pool(name="xg1", bufs=2) as xg1p,
            tc.tile_pool(name="sa0", bufs=1) as sa0p,
            tc.tile_pool(name="sa1", bufs=1) as sa1p,
            tc.tile_pool(name="small", bufs=4) as smp,
            tc.tile_pool(name="segps", bufs=2, space="PSUM") as seg_psp,
            tc.tile_pool(name="zps", bufs=2, space="PSUM") as z_psp,
            tc.tile_pool(name="aggps", bufs=2, space="PSUM") as agg_psp,
        ):
            # ---- resident loads (sync engine; scalar stays free) ----
            rr0_t = cpool.tile([P, G0], mybir.dt.bfloat16)
            nc.sync.dma_start(rr0_t[:], rr0_d.ap())
            rr1_t = cpool.tile([P, G1], mybir.dt.bfloat16)
            nc.sync.dma_start(rr1_t[:], rr1_d.ap())
            av0_t = cpool.tile([P, G0], mybir.dt.bfloat16)
            nc.sync.dma_start(av0_t[:], av0_d.ap())
            av1_t = cpool.tile([P, G1], mybir.dt.bfloat16)
            nc.sync.dma_start(av1_t[:], av1_d.ap())
            ixw0_t = cpool.tile([P, G0 * 8], mybir.dt.int16)
            nc.sync.dma_start(ixw0_t[:], ixw0_d.ap())
            ixw1_t = cpool.tile([P, G1 * 8], mybir.dt.int16)
            nc.sync.dma_start(ixw1_t[:], ixw1_d.ap())
            xofm_t = cpool.tile([DA, NPAD], mybir.dt.bfloat16)
            nc.sync.dma_start(xofm_t[:], xofm_d.ap())
            xonm_t = cpool.tile([P, NBLK, D], mybir.dt.float32)
            nc.sync.dma_start(
                xonm_t[:], bass.AP(xonm_d, 0, [[D, P], [P * D, NBLK], [1, D]])
            )
            wgi_t = cpool.tile([DA, D], mybir.dt.bfloat16)
            nc.sync.dma_start(wgi_t[:], wgi_d.ap())
            w2_t = cpool.tile([DA, D], mybir.dt.bfloat16)
            nc.sync.dma_start(w2_t[:], w2_d.ap())
            wn_t = cpool.tile([DA, D], mybir.dt.bfloat16)
            nc.sync.dma_start(wn_t[:], wn_d.ap())

            iota_i = cpool.tile([P, DW], mybir.dt.int32)
            nc.gpsimd.iota(iota_i[:], pattern=[[1, DW]], base=0, channel_multiplier=0)
            iota_b = cpool.tile([P, DW], mybir.dt.bfloat16)
            nc.vector.tensor_copy(iota_b[:], iota_i[:])
            # iota3[p, d, k] = d for k < KMAX: packed last dim for DVE 2x mode
            iota3 = cpool.tile([P, DW, KMAX], mybir.dt.bfloat16)
            for kk in range(KMAX):
                i3 = iota3[:]
                nc.vector.tensor_copy(
                    bass.AP(i3.tensor, i3.offset + kk, [i3.ap[0], [KMAX, DW]]),
                    iota_b[:],
                )

            aggx_t = cpool.tile([DA, NPAD], mybir.dt.bfloat16)   # feature-major aggX
            outb_t = cpool.tile([P, NBLK, D], mybir.dt.float32)

            # Tile's auto-sync does not cover prepare_only gather completion
            # (the deferred write is not tied to the DMA sem), so each
            # batch-half gets a "fence": a 1-column self-copy of the gathered
            # tile on DVE that carries explicit waits on the per-queue DMA
            # sems. The fence's write gives every downstream reader a real
            # data edge, and Tile's own WAR waits (PE ticks on later preps)
            # cover buffer reuse.
            qsems = [nc.alloc_semaphore(f"swdge_q{i}") for i in range(4)]
            qcum = [0, 0, 0, 0]
            qi = 0

            for bt in range(NBATCH):
                xg0_t = xg0p.tile([P, BPB * C0, ROWE], mybir.dt.bfloat16)
                xg1_t = xg1p.tile([P, BPB * C1, ROWE], mybir.dt.bfloat16)
                sa0_t = sa0p.tile([P, DW, BPB * C0], mybir.dt.bfloat16)
                sa1_t = sa1p.tile([P, DW, BPB * C1], mybir.dt.bfloat16)

                # 1) gather preps (Pool; ring backpressure paces the queues)
                fences = []
                for Ch, xg_t, ixw_t, tab in (
                    (C0, xg0_t, ixw0_t, xaug0_d),
                    (C1, xg1_t, ixw1_t, xaug1_d),
                ):
                    span = BPB * Ch * P          # idx count this batch-half
                    base = bt * span             # position in this half's stream
                    off = 0
                    while off < span:
                        n = min(MAXI, span - off)
                        s = base + off
                        q = qi % 4
                        nc.gpsimd.dma_gather(
                            out_ap=xg_t[:, off // P:(off + n) // P, :],
                            in_ap=tab.ap(),
                            idxs_ap=ixw_t[:, s // 16:(s + n) // 16],
                            num_idxs=n, num_idxs_reg=n, elem_size=ROWE,
                            prepare_only=True, sem=qsems[q], queue_num=q,
                        )
                        nc.gpsimd.trigger_dma(count=None, queue_num=q)
                        qcum[q] += 16
                        fences.append((xg_t, off // P, (off + n) // P, q, qcum[q]))
                        qi += 1
                        off += n

                # 2) S build (DVE, overlaps the gather DMA):
                #    [lane, dst, chunk] with packed last dims (2x_1p)
                for Ch, sa_t, rr_t, av_t in (
                    (C0, sa0_t, rr0_t, av0_t),
                    (C1, sa1_t, rr1_t, av1_t),
                ):
                    SPAN = BPB * Ch
                    g0 = bt * SPAN
                    sfull = sa_t[:]
                    for c_lo, c_hi in ((0, 4 * Ch), (4 * Ch, SPAN)):
                        sub = c_hi - c_lo
                        out_ap = bass.AP(sfull.tensor, sfull.offset + c_lo,
                                         [sfull.ap[0], [SPAN, DW], [1, sub]])
                        i3 = iota3[:]
                        iota_ap = bass.AP(i3.tensor, i3.offset,
                                          [i3.ap[0], [KMAX, DW], [0, sub // Ch], [1, Ch]])
                        rsl = rr_t[:, g0 + c_lo:g0 + c_hi]
                        rr_ap = bass.AP(rsl.tensor, rsl.offset,
                                        [rsl.ap[0], [0, DW], [1, sub]])
                        asl = av_t[:, g0 + c_lo:g0 + c_hi]
                        av_ap = bass.AP(asl.tensor, asl.offset,
                                        [asl.ap[0], [0, DW], [1, sub]])
                        nc.vector.tensor_tensor(out_ap, iota_ap, rr_ap,
                                                mybir.AluOpType.is_equal)
                        nc.vector.tensor_tensor(out_ap, out_ap, av_ap,
                                                mybir.AluOpType.mult)

                # 3) fences (DVE, after the S builds): 1-col self-copy per prep
                #    carrying that queue's DMA-completion wait (1 wait slot)
                for xg_t, c_a, c_b, q, tgt in fences:
                    fence = nc.vector.tensor_copy(xg_t[:, c_a:c_b, 0:1],
                                                  xg_t[:, c_a:c_b, 0:1])
                    fence._wait_ge(qsems[q], tgt)

                for lb in range(BPB):
                    j = bt * BPB + lb
                    # full-bank (2KB) psum tiles: accumulation-group zero
                    # regions are bank-granular, so tiles must not share banks
                    seg_ps = seg_psp.tile([P, 512], mybir.dt.float32, space="PSUM")
                    ci = 0
                    for Ch, xg_t, sa_t in ((C0, xg0_t, sa0_t), (C1, xg1_t, sa1_t)):
                        SPAN = BPB * Ch
                        sfull = sa_t[:]
                        for c in range(Ch):
                            lc = lb * Ch + c
                            rhs = bass.AP(sfull.tensor, sfull.offset + lc,
                                          [sfull.ap[0], [SPAN, DW]])
                            nc.tensor.matmul(
                                out=seg_ps[:, :DW],
                                lhsT=xg_t[:, lc, :],
                                rhs=rhs,
                                start=(ci == 0),
                                stop=(ci == CPB - 1),
                            )
                            ci += 1
                    nc.scalar.copy(aggx_t[:, j * P:(j + 1) * P], seg_ps[:DA, :P])

                    # dense tail for this block of 128 nodes
                    z_ps = z_psp.tile([P, 512], mybir.dt.float32, space="PSUM")
                    agg_ps = agg_psp.tile([P, 512], mybir.dt.float32, space="PSUM")
                    sl = slice(j * DW, (j + 1) * DW)
                    nc.tensor.matmul(out=z_ps[:, :D], lhsT=xofm_t[:, sl],
                                     rhs=wgi_t[:], start=True, stop=False)
                    nc.tensor.matmul(out=z_ps[:, :D], lhsT=aggx_t[:, sl],
                                     rhs=w2_t[:], start=False, stop=True)
                    nc.tensor.matmul(out=agg_ps[:, :D], lhsT=aggx_t[:, sl],
                                     rhs=wn_t[:], start=True, stop=True)
                    g1_t = smp.tile([P, D], mybir.dt.float32)
                    nc.scalar.activation(g1_t[:], z_ps[:, :D], mybir.ActivationFunctionType.Sigmoid)
                    g2_t = smp.tile([P, D], mybir.dt.float32)
                    nc.scalar.activation(g2_t[:], z_ps[:, :D], mybir.ActivationFunctionType.Sigmoid,
                                         scale=-1.0)
                    nc.vector.tensor_tensor(g1_t[:], agg_ps[:, :D], g1_t[:], mybir.AluOpType.mult)
                    nc.vector.tensor_tensor(g2_t[:], xonm_t[:, j, :], g2_t[:], mybir.AluOpType.mult)
                    nc.vector.tensor_add(outb_t[:, j, :], g1_t[:], g2_t[:])

            nc.sync.dma_start(
                bass.AP(y_d, 0, [[D, P], [P * D, NBLK], [1, D]]), outb_t[:]
            )

    nc.compile()
    return nc


# test-harness hooks: set TRACE_TMPDIR to capture an NTFF profile on the next
# call; LAST_EXEC_NS then holds the profiled kernel execution time.
TRACE_TMPDIR = None
LAST_EXEC_NS = None


def kernel(X, a_vals, Wn, bn, Wgi, bgi, Wgn, bgn, row, col):
    global LAST_EXEC_NS
    from concourse.bass_utils import run_bass_kernel_spmd

    per_core, (C0, C1) = _host_prep(X, a_vals, Wn, bn, Wgi, bgi, Wgn, bgn, row, col)
    if (C0, C1) not in _prog_cache:
        _prog_cache[(C0, C1)] = _build_program(C0, C1)
    nc = _prog_cache[(C0, C1)]

    kwargs = {}
    if TRACE_TMPDIR is not None:
        kwargs = {"trace": True, "tmpdir": TRACE_TMPDIR}
    res = run_bass_kernel_spmd(nc, per_core, core_ids=list(range(NC_)), **kwargs)
    LAST_EXEC_NS = res.exec_time_ns
    out = np.empty((N, D), np.float32)
    for k in range(NC_):
        out[k * NSH:(k + 1) * NSH] = res.results[k]["y"][:NSH]
    return out


# revision 21
# speedup vs baseline: 3.9065x; 3.9065x over previous
"""GatedGCN message-passing kernel for 8 Trainium2 NeuronCores (Bass/Tile).

Math (reference):
    newX = X @ Wn + bn
    agg  = segment_sum(a_vals[:,None] * newX[col], row, N)
    gate = sigmoid(X @ Wgi + bgi + agg @ Wgn + bgn)
    out  = agg * gate + X * (1 - gate)

Device strategy (per core, destination-sharded edges):
    Linearity lets the dense projection move past the aggregation:
        agg = (segsum(a * X_aug[col])) @ Wn_aug,    X_aug = [X | 1], Wn_aug = [Wn; bn]
    so the gather runs on raw bf16 X rows (SWDGE dma_gather, 256B rows) and the
    segment-sum is computed as one-hot matmuls accumulating in PSUM:
        per 128-edge chunk c of a 128-destination block b:
            S[e, d]    = (iota[d] == row_rel[e]) * a[e]          (DVE, 2 ops)
            aggX[:, b] += Xg_c.T @ S_c                            (PE, PSUM accum)
    giving aggX feature-major [97, dst]. All downstream dense math is then
        z    = Xo_aug @ Wgi_aug + aggX_aug @ (Wn_aug @ Wgn)      (biases folded)
        agg  = aggX_aug @ Wn_aug
        out  = agg * sigmoid(z) + Xo * sigmoid(-z)
    dma_gather uses int16 indices, so the node table is split in two 25000-row
    halves; per-half edge streams are block-major contiguous so every gather
    prep covers a contiguous index span.

Perf structure (v2):
      - gathers use prepare_only=True + trigger_dma: the Pool engine only
        writes descriptors (~0.34ns each) into a 2048-entry SWDGE ring
        (dynamic_dma_scratch_size=32768) over 4 queues; the 16 DMA engines
        drain asynchronously. This removes the Pool-engine hold that
        serialized the baseline.
      - the selection matrix S is built in [lane, dst, chunk] layout against a
        materialized iota3[P, DW, K] so every DVE operand has a packed
        (stride-1, 2-byte) last dim -> DVE 2x_1p mode (2 elem/cycle/lane).
      - resident loads ride the Sync engine, keeping Scalar for
        sigmoids + PSUM->SBUF copies.
"""

import os

# Reset NeuronCores on runtime open: residual device state from prior runs
# otherwise degrades kernel time by ~15%. Must be set before NRT init.
os.environ.setdefault("NEURON_RT_RESET_CORES", "1")

import numpy as np
import ml_dtypes

N = 50000
E = 800000
D = 96
DA = D + 1          # augmented features (ones column)
ROWE = 128          # padded row elements in the gather table (256B bf16)
NC_ = 8
NSH = N // NC_      # 6250 nodes per core
NHALF = N // 2      # table split for int16 gather indices
DW = 128            # destinations per block
NBLK = (NSH + DW - 1) // DW          # 49 blocks per core
NPAD = NBLK * DW                     # 6272 padded nodes per core
P = 128                              # edges per chunk
BPB = 7                              # blocks per batch
NBATCH = NBLK // BPB                 # 7 batches
MAXI = 1024                          # gather idxs per prep (SWDGE ring cap)

_bf16 = ml_dtypes.bfloat16

_prog_cache = {}


def _host_prep(X, a_vals, Wn, bn, Wgi, bgi, Wgn, bgn, row, col):
    X = np.asarray(X, np.float32)
    a_vals = np.asarray(a_vals, np.float32)
    row = np.asarray(row, np.int64)
    col = np.asarray(col, np.int64)

    Wn_aug = np.vstack([np.asarray(Wn, np.float32), np.asarray(bn, np.float32)[None, :]])
    Wgi_aug = np.vstack([np.asarray(Wgi, np.float32),
                         (np.asarray(bgi, np.float32) + np.asarray(bgn, np.float32))[None, :]])
    W2_aug = Wn_aug @ np.asarray(Wgn, np.float32)

    X_pad = np.zeros((N, ROWE), np.float32)
    X_pad[:, :D] = X
    X_pad[:, D] = 1.0
    X_pad_bf = X_pad.astype(_bf16)
    xaug0 = np.ascontiguousarray(X_pad_bf[:NHALF])
    xaug1 = np.ascontiguousarray(X_pad_bf[NHALF:])

    core = row // NSH
    local = row - core * NSH
    blk = local // DW
    rr = local - blk * DW
    half = (col >= NHALF).astype(np.int64)

    # group edges by (core, block, half); get within-group positions
    gkey = (core * NBLK + blk) * 2 + half
    order = np.argsort(gkey, kind="stable")
    gk_sorted = gkey[order]
    counts = np.bincount(gk_sorted, minlength=NC_ * NBLK * 2)
    starts = np.concatenate([[0], np.cumsum(counts)])
    pos = np.arange(E, dtype=np.int64) - starts[gk_sorted]

    cnt2 = counts.reshape(NC_ * NBLK, 2)
    C0 = int(np.ceil(cnt2[:, 0].max() / P))
    C1 = int(np.ceil(cnt2[:, 1].max() / P))

    # per-half slot: block-major chunks, so batches cover contiguous spans
    b_all = blk[order]
    h_all = half[order]
    c_all = pos // P
    lane_all = pos - c_all * P
    ch_all = np.where(h_all == 0, C0, C1)
    slot_all = (b_all * ch_all + c_all) * P + lane_all   # within-half slot

    col_l = (col - half * NHALF)[order].astype(np.int32)   # local table row
    rr_o = rr[order].astype(np.float32)
    av_o = a_vals[order].astype(np.float32)
    core_o = core[order]

    G0 = NBLK * C0
    G1 = NBLK * C1

    per_core = []
    for k in range(NC_):
        d = {}
        for h, G in ((0, G0), (1, G1)):
            m = (core_o == k) & (h_all == h)
            slot = slot_all[m]
            idx_arr = np.zeros(G * P, np.int32)
            rr_arr = np.zeros(G * P, np.float32)
            av_arr = np.zeros(G * P, np.float32)
            idx_arr[slot] = col_l[m]
            rr_arr[slot] = rr_o[m]
            av_arr[slot] = av_o[m]

            d[f"rr{h}"] = np.ascontiguousarray(rr_arr.reshape(G, P).T).astype(_bf16)
            d[f"av{h}"] = np.ascontiguousarray(av_arr.reshape(G, P).T).astype(_bf16)
            # gather index stream wrapped in 16 partitions, replicated x8
            ix = idx_arr.reshape(G, P).astype(np.int16)
            wr = ix.reshape(G * P // 16, 16).T                 # [16, G*8]
            d[f"ixw{h}"] = np.ascontiguousarray(np.tile(wr, (8, 1)))   # [128, G*8]

        xo = np.zeros((NPAD, DA), np.float32)
        xo[:NSH] = X_pad[k * NSH:(k + 1) * NSH, :DA]
        d["xofm"] = np.ascontiguousarray(xo.T).astype(_bf16)          # [97, 6272]
        d["xonm"] = np.ascontiguousarray(xo[:, :D])                   # [6272, 96] f32
        d["xaug0"] = xaug0
        d["xaug1"] = xaug1
        d["wgi"] = Wgi_aug.astype(_bf16)
        d["w2"] = W2_aug.astype(_bf16)
        d["wn"] = Wn_aug.astype(_bf16)
        per_core.append(d)
    return per_core, (C0, C1)


def _build_program(C0, C1):
    import concourse.bass as bass
    import concourse.tile as tile
    from concourse import bacc, mybir

    CPB = C0 + C1
    G0 = NBLK * C0
    G1 = NBLK * C1
    KMAX = max(C0, C1)

    nc = bacc.Bacc("TRN2", target_bir_lowering=False, debug=False, num_devices=NC_,
                   num_swdge_queues=4)

    xaug0_d = nc.dram_tensor("xaug0", [NHALF, ROWE], mybir.dt.bfloat16, kind="ExternalInput")
    xaug1_d = nc.dram_tensor("xaug1", [NHALF, ROWE], mybir.dt.bfloat16, kind="ExternalInput")
    ixw0_d = nc.dram_tensor("ixw0", [P, G0 * 8], mybir.dt.int16, kind="ExternalInput")
    ixw1_d = nc.dram_tensor("ixw1", [P, G1 * 8], mybir.dt.int16, kind="ExternalInput")
    rr0_d = nc.dram_tensor("rr0", [P, G0], mybir.dt.bfloat16, kind="ExternalInput")
    rr1_d = nc.dram_tensor("rr1", [P, G1], mybir.dt.bfloat16, kind="ExternalInput")
    av0_d = nc.dram_tensor("av0", [P, G0], mybir.dt.bfloat16, kind="ExternalInput")
    av1_d = nc.dram_tensor("av1", [P, G1], mybir.dt.bfloat16, kind="ExternalInput")
    xofm_d = nc.dram_tensor("xofm", [DA, NPAD], mybir.dt.bfloat16, kind="ExternalInput")
    xonm_d = nc.dram_tensor("xonm", [NPAD, D], mybir.dt.float32, kind="ExternalInput")
    wgi_d = nc.dram_tensor("wgi", [DA, D], mybir.dt.bfloat16, kind="ExternalInput")
    w2_d = nc.dram_tensor("w2", [DA, D], mybir.dt.bfloat16, kind="ExternalInput")
    wn_d = nc.dram_tensor("wn", [DA, D], mybir.dt.bfloat16, kind="ExternalInput")
    y_d = nc.dram_tensor("y", [NPAD, D], mybir.dt.float32, kind="ExternalOutput")

    with tile.TileContext(nc) as tc:
        with (
            tc.tile_pool(name="const", bufs=1) as cpool,
            tc.tile_pool(name="xg0", bufs=2) as xg0p,
            tc.tile_pool(name="xg1", bufs=2) as xg1p,
            tc.tile_pool(name="sa0", bufs=1) as sa0p,
            tc.tile_pool(name="sa1", bufs=1) as sa1p,
            tc.tile_pool(name="small", bufs=4) as smp,
            tc.tile_pool(name="segps", bufs=2, space="PSUM") as seg_psp,
            tc.tile_pool(name="zps", bufs=2, space="PSUM") as z_psp,
            tc.tile_pool(name="aggps", bufs=2, space="PSUM") as agg_psp,
        ):
            # ---- resident loads (sync engine; scalar stays free) ----
            rr0_t = cpool.tile([P, G0], mybir.dt.bfloat16)
            nc.sync.dma_start(rr0_t[:], rr0_d.ap())
            rr1_t = cpool.tile([P, G1], mybir.dt.bfloat16)
            nc.sync.dma_start(rr1_t[:], rr1_d.ap())
            av0_t = cpool.tile([P, G0], mybir.dt.bfloat16)
            nc.sync.dma_start(av0_t[:], av0_d.ap())
            av1_t = cpool.tile([P, G1], mybir.dt.bfloat16)
            nc.sync.dma_start(av1_t[:], av1_d.ap())
            ixw0_t = cpool.tile([P, G0 * 8], mybir.dt.int16)
            nc.sync.dma_start(ixw0_t[:], ixw0_d.ap())
            ixw1_t = cpool.tile([P, G1 * 8], mybir.dt.int16)
            nc.sync.dma_start(ixw1_t[:], ixw1_d.ap())
            xofm_t = cpool.tile([DA, NPAD], mybir.dt.bfloat16)
            nc.sync.dma_start(xofm_t[:], xofm_d.ap())
            xonm_t = cpool.tile([P, NBLK, D], mybir.dt.float32)
            nc.sync.dma_start(
                xonm_t[:], bass.AP(xonm_d, 0, [[D, P], [P * D, NBLK], [1, D]])
            )
            wgi_t = cpool.tile([DA, D], mybir.dt.bfloat16)
            nc.sync.dma_start(wgi_t[:], wgi_d.ap())
            w2_t = cpool.tile([DA, D], mybir.dt.bfloat16)
            nc.sync.dma_start(w2_t[:], w2_d.ap())
            wn_t = cpool.tile([DA, D], mybir.dt.bfloat16)
            nc.sync.dma_start(wn_t[:], wn_d.ap())

            iota_i = cpool.tile([P, DW], mybir.dt.int32)
            nc.gpsimd.iota(iota_i[:], pattern=[[1, DW]], base=0, channel_multiplier=0)
            iota_b = cpool.tile([P, DW], mybir.dt.bfloat16)
            nc.vector.tensor_copy(iota_b[:], iota_i[:])
            # iota3[p, d, k] = d for k < KMAX: packed last dim for DVE 2x mode
            iota3 = cpool.tile([P, DW, KMAX], mybir.dt.bfloat16)
            for kk in range(KMAX):
                i3 = iota3[:]
                nc.vector.tensor_copy(
                    bass.AP(i3.tensor, i3.offset + kk, [i3.ap[0], [KMAX, DW]]),
                    iota_b[:],
                )

            aggx_t = cpool.tile([DA, NPAD], mybir.dt.bfloat16)   # feature-major aggX
            outb_t = cpool.tile([P, NBLK, D], mybir.dt.float32)

            qi = 0

            for bt in range(NBATCH):
                xg0_t = xg0p.tile([P, BPB * C0, ROWE], mybir.dt.bfloat16)
                xg1_t = xg1p.tile([P, BPB * C1, ROWE], mybir.dt.bfloat16)
                sa0_t = sa0p.tile([P, DW, BPB * C0], mybir.dt.bfloat16)
                sa1_t = sa1p.tile([P, DW, BPB * C1], mybir.dt.bfloat16)

                # 1) gathers (gen_mode=0; the Pool engine frees after desc-gen
                #    enqueue, and the 4 queue contexts generate in parallel —
                #    measured 2.2 ns/descriptor at 4-queue steady state)
                for Ch, xg_t, ixw_t, tab in (
                    (C0, xg0_t, ixw0_t, xaug0_d),
                    (C1, xg1_t, ixw1_t, xaug1_d),
                ):
                    span = BPB * Ch * P          # idx count this batch-half
                    base = bt * span             # position in this half's stream
                    off = 0
                    while off < span:
                        n = min(MAXI, span - off)
                        s = base + off
                        nc.gpsimd.dma_gather(
                            out_ap=xg_t[:, off // P:(off + n) // P, :],
                            in_ap=tab.ap(),
                            idxs_ap=ixw_t[:, s // 16:(s + n) // 16],
                            num_idxs=n, num_idxs_reg=n, elem_size=ROWE,
                            queue_num=qi % 4,
                        )
                        qi += 1
                        off += n

                # 2) S build (DVE, overlaps the gather DMA):
                #    [lane, dst, chunk] with packed last dims (2x_1p)
                for Ch, sa_t, rr_t, av_t in (
                    (C0, sa0_t, rr0_t, av0_t),
                    (C1, sa1_t, rr1_t, av1_t),
                ):
                    SPAN = BPB * Ch
                    g0 = bt * SPAN
                    sfull = sa_t[:]
                    for c_lo, c_hi in ((0, 4 * Ch), (4 * Ch, SPAN)):
                        sub = c_hi - c_lo
                        out_ap = bass.AP(sfull.tensor, sfull.offset + c_lo,
                                         [sfull.ap[0], [SPAN, DW], [1, sub]])
                        i3 = iota3[:]
                        iota_ap = bass.AP(i3.tensor, i3.offset,
                                          [i3.ap[0], [KMAX, DW], [0, sub // Ch], [1, Ch]])
                        rsl = rr_t[:, g0 + c_lo:g0 + c_hi]
                        rr_ap = bass.AP(rsl.tensor, rsl.offset,
                                        [rsl.ap[0], [0, DW], [1, sub]])
                        asl = av_t[:, g0 + c_lo:g0 + c_hi]
                        av_ap = bass.AP(asl.tensor, asl.offset,
                                        [asl.ap[0], [0, DW], [1, sub]])
                        nc.vector.tensor_tensor(out_ap, iota_ap, rr_ap,
                                                mybir.AluOpType.is_equal)
                        nc.vector.tensor_tensor(out_ap, out_ap, av_ap,
                                                mybir.AluOpType.mult)

                for lb in range(BPB):
                    j = bt * BPB + lb
                    # full-bank (2KB) psum tiles: accumulation-group zero
                    # regions are bank-granular, so tiles must not share banks
                    seg_ps = seg_psp.tile([P, 512], mybir.dt.float32, space="PSUM")
                    ci = 0
                    for Ch, xg_t, sa_t in ((C0, xg0_t, sa0_t), (C1, xg1_t, sa1_t)):
                        SPAN = BPB * Ch
                        sfull = sa_t[:]
                        for c in range(Ch):
                            lc = lb * Ch + c
                            rhs = bass.AP(sfull.tensor, sfull.offset + lc,
                                          [sfull.ap[0], [SPAN, DW]])
                            nc.tensor.matmul(
                                out=seg_ps[:, :DW],
                                lhsT=xg_t[:, lc, :],
                                rhs=rhs,
                                start=(ci == 0),
                                stop=(ci == CPB - 1),
                            )
                            ci += 1
                    nc.scalar.copy(aggx_t[:, j * P:(j + 1) * P], seg_ps[:DA, :P])

                    # dense tail for this block of 128 nodes
                    z_ps = z_psp.tile([P, 512], mybir.dt.float32, space="PSUM")
                    agg_ps = agg_psp.tile([P, 512], mybir.dt.float32, space="PSUM")
                    sl = slice(j * DW, (j + 1) * DW)
                    nc.tensor.matmul(out=z_ps[:, :D], lhsT=xofm_t[:, sl],
                                     rhs=wgi_t[:], start=True, stop=False)
                    nc.tensor.matmul(out=z_ps[:, :D], lhsT=aggx_t[:, sl],
                                     rhs=w2_t[:], start=False, stop=True)
                    nc.tensor.matmul(out=agg_ps[:, :D], lhsT=aggx_t[:, sl],
                                     rhs=wn_t[:], start=True, stop=True)
                    g1_t = smp.tile([P, D], mybir.dt.float32)
                    nc.scalar.activation(g1_t[:], z_ps[:, :D], mybir.ActivationFunctionType.Sigmoid)
                    g2_t = smp.tile([P, D], mybir.dt.float32)
                    nc.scalar.activation(g2_t[:], z_ps[:, :D], mybir.ActivationFunctionType.Sigmoid,
                                         scale=-1.0)
                    nc.vector.tensor_tensor(g1_t[:], agg_ps[:, :D], g1_t[:], mybir.AluOpType.mult)
                    nc.vector.tensor_tensor(g2_t[:], xonm_t[:, j, :], g2_t[:], mybir.AluOpType.mult)
                    nc.vector.tensor_add(outb_t[:, j, :], g1_t[:], g2_t[:])

            nc.sync.dma_start(
                bass.AP(y_d, 0, [[D, P], [P * D, NBLK], [1, D]]), outb_t[:]
            )

    nc.compile()
    return nc


# test-harness hooks: set TRACE_TMPDIR to capture an NTFF profile on the next
# call; LAST_EXEC_NS then holds the profiled kernel execution time.
TRACE_TMPDIR = None
LAST_EXEC_NS = None


def kernel(X, a_vals, Wn, bn, Wgi, bgi, Wgn, bgn, row, col):
    global LAST_EXEC_NS
    from concourse.bass_utils import run_bass_kernel_spmd

    per_core, (C0, C1) = _host_prep(X, a_vals, Wn, bn, Wgi, bgi, Wgn, bgn, row, col)
    if (C0, C1) not in _prog_cache:
        _prog_cache[(C0, C1)] = _build_program(C0, C1)
    nc = _prog_cache[(C0, C1)]

    kwargs = {}
    if TRACE_TMPDIR is not None:
        kwargs = {"trace": True, "tmpdir": TRACE_TMPDIR}
    res = run_bass_kernel_spmd(nc, per_core, core_ids=list(range(NC_)), **kwargs)
    LAST_EXEC_NS = res.exec_time_ns
    out = np.empty((N, D), np.float32)
    for k in range(NC_):
        out[k * NSH:(k + 1) * NSH] = res.results[k]["y"][:NSH]
    return out
